# revision 15
# baseline (speedup 1.0000x reference)
"""Trainium2 Bass kernel for nn_AttnRnnModel (GRU + history attention + big output).

Sharding (8 cores):
  - batch-sharded (8 batches/core): embedding gathers, gx precompute, GRU
    recurrence, history pooling + attention context
  - AllGather of [last|context|last] (8x1536 -> 64x1536)
  - U-sharded final matmul (625 vocab columns/core) + broadcast-over-T output
    write of the core's [64, 128, 625] slice; host concatenates along U.

All heavy matmuls run as float32r (full-rate on PE, ~1e-5 rel precision).
Weight transposes (W_ih.T etc.) are done host-side; data-dependent
transposes (gathered embeddings, h_t, attention features) on the PE.
"""

import numpy as np
import concourse.bass as bass
import concourse.bacc as bacc
import concourse.mybir as mybir
from concourse import tile
from concourse.bass_utils import run_bass_kernel_spmd
from concourse.masks import make_identity

F32 = mybir.dt.float32
F32R = mybir.dt.float32r
I32 = mybir.dt.int32
AF = mybir.ActivationFunctionType
ALU = mybir.AluOpType
AX = mybir.AxisListType

# problem constants (hardcoded per harness contract)
LOC_SIZE, LOC_EMB = 100000, 256
TIM_SIZE, TIM_EMB = 48, 64
UID_SIZE, UID_EMB = 5000, 128
HID = 512
B, T = 64, 128
L_HIST, G = 1024, 4
NCORES = 8
BL = B // NCORES              # 8 batches per core
NG = L_HIST // G              # 256 history groups per batch
NGL = BL * NG                 # 2048 groups per core
DIN = LOC_EMB + TIM_EMB       # 320
DH = LOC_EMB + TIM_EMB + UID_EMB  # 448
H3 = 3 * HID                  # 1536
UL = UID_SIZE // NCORES       # 625 vocab columns per core
ULP = UL + 1                  # padded even width for fp32r matmuls
TOK = BL * T                  # 1024 current-traj tokens per core

_STATE = {}


def _build(sim=False, gru_steps=T, skip_hist=False, skip_p1=False):
    nc = bacc.Bacc("TRN2", target_bir_lowering=False, debug=False,
                   num_devices=NCORES)

    # ---- per-core external inputs (host pre-arranged) ----
    idx_cur = nc.declare_dram_parameter("idx_cur", [128, TOK // 128], I32, isOutput=False)
    idx_tim = nc.declare_dram_parameter("idx_tim", [128, TOK // 128], I32, isOutput=False)
    idx_hloc = nc.declare_dram_parameter("idx_hloc", [128, NGL // 128], I32, isOutput=False)
    idx_htim = nc.declare_dram_parameter("idx_htim", [128, NGL // 128], I32, isOutput=False)
    idx_huid = nc.declare_dram_parameter("idx_huid", [128, BL * L_HIST // 128], I32, isOutput=False)
    idx_last = nc.declare_dram_parameter("idx_last", [BL, 1], I32, isOutput=False)

    emb_loc = nc.declare_dram_parameter("emb_loc", [LOC_SIZE, LOC_EMB], F32, isOutput=False)
    emb_tim = nc.declare_dram_parameter("emb_tim", [TIM_SIZE, TIM_EMB], F32, isOutput=False)
    emb_uid = nc.declare_dram_parameter("emb_uid", [UID_SIZE, UID_EMB], F32, isOutput=False)

    wiht = nc.declare_dram_parameter("wiht", [DIN + 1, H3], F32R, isOutput=False)   # [W_ih.T; b_ih]
    whht = nc.declare_dram_parameter("whht", [HID, H3], F32R, isOutput=False)       # W_hh.T
    bhh = nc.declare_dram_parameter("bhh", [1, H3], F32R, isOutput=False)
    watt = nc.declare_dram_parameter("watt", [DH + 1, HID], F32R, isOutput=False)   # [W_attn.T; b_attn]
    wfin = nc.declare_dram_parameter("wfin", [H3 + 1, ULP], F32R, isOutput=False)    # [W_final.T slice; b_final slice]
    smean = nc.declare_dram_parameter("smean", [128, 32], F32, isOutput=False)      # group-mean matrix
    onesd = nc.declare_dram_parameter("onesd", [1, 128], F32R, isOutput=False)
    eye8 = nc.declare_dram_parameter("eye8", [BL, BL], F32R, isOutput=False)

    score = nc.declare_dram_parameter("score", [B, T, UL], F32, isOutput=True)

    # ---- internal DRAM ----
    gx_d = nc.dram_tensor("gx_d", [T, BL, H3], F32R)
    outs_d = nc.dram_tensor("outs_d", [T * BL, HID], F32)
    og_local = nc.dram_tensor("og_local", [BL, H3], F32)
    og_shared = nc.dram_tensor("og_shared", [B, H3], F32, addr_space="Shared")

    FEAT_CH = ((0, 128), (128, 128), (256, 128), (384, 65))

    with tile.TileContext(nc) as tc:
        with (
            tc.tile_pool(name="const", bufs=1) as cpool,
            tc.tile_pool(name="wts", bufs=1) as wpool,
            tc.tile_pool(name="state", bufs=1) as spool,
            tc.tile_pool(name="work", bufs=2) as work,
            tc.tile_pool(name="gxt", bufs=3) as gxtp,
            tc.tile_pool(name="ps_g", bufs=4, space="PSUM") as ps_g,
            tc.tile_pool(name="ps_t", bufs=2, space="PSUM") as ps_t,
            tc.tile_pool(name="ps_b", bufs=2, space="PSUM") as ps_b,
        ):
            ident = cpool.tile([128, 128], F32, tag="ident")
            make_identity(nc, ident[:])
            ones1r = cpool.tile([1, 128], F32R, tag="ones1r")
            nc.sync.dma_start(out=ones1r[:], in_=onesd[:])
            i8 = cpool.tile([BL, BL], F32R, tag="i8")
            nc.sync.dma_start(out=i8[:], in_=eye8[:])

            # ---- load persistent weights ----
            whht_sb = []
            for k in range(4):
                t_ = wpool.tile([128, H3], F32R, tag=f"whht{k}")
                nc.sync.dma_start(out=t_[:], in_=whht[128 * k:128 * (k + 1), :])
                whht_sb.append(t_)
            bhh_sb = wpool.tile([1, H3], F32R, tag="bhh")
            nc.sync.dma_start(out=bhh_sb[:], in_=bhh[:])
            watt_sb = []
            for k, (p0, pn) in enumerate(FEAT_CH):
                t_ = wpool.tile([pn, HID], F32R, tag=f"watt{k}")
                nc.sync.dma_start(out=t_[:], in_=watt[p0:p0 + pn, :])
                watt_sb.append(t_)
            wfin_sb = []
            for k in range(12):
                t_ = wpool.tile([128, ULP], F32R, tag=f"wfin{k}")
                nc.sync.dma_start(out=t_[:], in_=wfin[128 * k:128 * (k + 1), :])
                wfin_sb.append(t_)
            wfin_b = wpool.tile([1, ULP], F32R, tag="wfinb")
            nc.sync.dma_start(out=wfin_b[:], in_=wfin[H3:H3 + 1, :])
            smean_sb = cpool.tile([128, 32], F32, tag="smean")
            nc.sync.dma_start(out=smean_sb[:], in_=smean[:])

            # ---- load index tiles ----
            def idx_tile(name, src, ncol):
                t_ = cpool.tile([128, ncol], I32, tag=name)
                nc.sync.dma_start(out=t_[:], in_=src[:])
                return t_
            ixc = idx_tile("ixc", idx_cur, TOK // 128)
            ixt = idx_tile("ixt", idx_tim, TOK // 128)
            ixhl = idx_tile("ixhl", idx_hloc, NGL // 128)
            ixht = idx_tile("ixht", idx_htim, NGL // 128)
            ixhu = idx_tile("ixhu", idx_huid, BL * L_HIST // 128)
            ixl = cpool.tile([BL, 1], I32, tag="ixl")
            nc.sync.dma_start(out=ixl[:], in_=idx_last[:])

            # histT[m] = tanh(featT.T-ish @ W_attn) in [HID-chunk, groups] layout
            histT = []
            for m in range(4):
                t_ = wpool.tile([128, NGL], F32R, tag=f"histT{m}")
                histT.append(t_)

            # ================= phase 1: x gather + transpose + gx =================
            with (tc.tile_pool(name="p1", bufs=2) as p1,
                  tc.tile_pool(name="p1w", bufs=1) as p1w):
                wiht_sb = []
                for k, (p0, pn) in enumerate(((0, 128), (128, 128), (256, 65))):
                    wk = p1w.tile([pn, H3], F32R, tag=f"wih{k}")
                    nc.sync.dma_start(out=wk[:], in_=wiht[p0:p0 + pn, :])
                    wiht_sb.append(wk)
                for b in range(0 if skip_p1 else BL):
                    x_t = p1.tile([128, DIN], F32, tag="x_t")
                    nc.gpsimd.indirect_dma_start(
                        out=x_t[:, 0:LOC_EMB], out_offset=None, in_=emb_loc[:],
                        in_offset=bass.IndirectOffsetOnAxis(ap=ixc[:, b:b + 1], axis=0))
                    nc.gpsimd.indirect_dma_start(
                        out=x_t[:, LOC_EMB:DIN], out_offset=None, in_=emb_tim[:],
                        in_offset=bass.IndirectOffsetOnAxis(ap=ixt[:, b:b + 1], axis=0))
                    xT = []
                    for k, (c0, cn) in enumerate(((0, 128), (128, 128), (256, 64))):
                        pst = ps_t.tile([128, 128], F32, tag="pst")
                        nc.tensor.transpose(out=pst[:cn, :128], in_=x_t[:, c0:c0 + cn],
                                            identity=ident[:])
                        xk = p1.tile([cn + (1 if k == 2 else 0), 128], F32R, tag=f"xT{k}")
                        nc.vector.tensor_copy(xk[:cn, :], pst[:cn, :128])
                        if k == 2:
                            nc.sync.dma_start(out=xk[cn:cn + 1, :], in_=onesd[:])
                        xT.append(xk)
                    for n in range(3):
                        psg = ps_b.tile([128, HID], F32, tag="psb")
                        for k in range(3):
                            nc.tensor.matmul(
                                out=psg[:], lhsT=xT[k][:],
                                rhs=wiht_sb[k][:, HID * n:HID * (n + 1)],
                                start=(k == 0), stop=(k == 2))
                        gxc = p1.tile([128, HID], F32R, tag="gxc")
                        nc.vector.tensor_copy(gxc[:], psg[:])
                        nc.sync.dma_start(out=gx_d[:, b, HID * n:HID * (n + 1)],
                                          in_=gxc[:])

            # ================= phase 2: history features -> histT =================
            with tc.tile_pool(name="p2", bufs=2) as p2:
                for gc in range(0 if skip_hist else NGL // 512):           # 4 chunks of 512 groups
                    fTc = []
                    for k, (c0, cn) in enumerate(FEAT_CH):
                        t_ = p2.tile([cn, 512], F32R, tag=f"fTc{k}")
                        fTc.append(t_)
                    for fi in range(4):                # 4 feat tiles of 128 groups
                        f = 4 * gc + fi
                        ft = p2.tile([128, DH + 1], F32, tag="ft")
                        nc.gpsimd.indirect_dma_start(
                            out=ft[:, 0:LOC_EMB], out_offset=None, in_=emb_loc[:],
                            in_offset=bass.IndirectOffsetOnAxis(ap=ixhl[:, f:f + 1], axis=0))
                        nc.gpsimd.indirect_dma_start(
                            out=ft[:, LOC_EMB:DIN], out_offset=None, in_=emb_tim[:],
                            in_offset=bass.IndirectOffsetOnAxis(ap=ixht[:, f:f + 1], axis=0))
                        for q in range(4):
                            ut = p2.tile([128, UID_EMB], F32, tag="ut")
                            nc.gpsimd.indirect_dma_start(
                                out=ut[:], out_offset=None, in_=emb_uid[:],
                                in_offset=bass.IndirectOffsetOnAxis(
                                    ap=ixhu[:, 4 * f + q:4 * f + q + 1], axis=0))
                            psm = ps_t.tile([128, 128], F32, tag="pst")
                            nc.tensor.matmul(out=psm[:32, :UID_EMB], lhsT=smean_sb[:],
                                             rhs=ut[:], start=True, stop=True)
                            nc.vector.tensor_copy(ft[32 * q:32 * (q + 1), DIN:DH],
                                                  psm[:32, :UID_EMB])
                        nc.gpsimd.memset(ft[:, DH:DH + 1], 1.0)
                        for k, (c0, cn) in enumerate(FEAT_CH):
                            pst = ps_t.tile([128, 128], F32, tag="pst")
                            nc.tensor.transpose(out=pst[:cn, :128],
                                                in_=ft[:, c0:c0 + cn],
                                                identity=ident[:])
                            nc.vector.tensor_copy(fTc[k][:, 128 * fi:128 * (fi + 1)],
                                                  pst[:cn, :128])
                    for m in range(4):
                        psh = ps_b.tile([128, 512], F32, tag="psb")
                        for k in range(4):
                            nc.tensor.matmul(
                                out=psh[:], lhsT=watt_sb[k][:, 128 * m:128 * (m + 1)],
                                rhs=fTc[k][:], start=(k == 0), stop=(k == 3))
                        nc.scalar.activation(histT[m][:, 512 * gc:512 * (gc + 1)],
                                             psh[:], AF.Tanh)

            # ================= phase 3: GRU over T steps =================
            hT = []
            zinit = spool.tile([128, BL], F32, tag="zinit")
            nc.gpsimd.memset(zinit[:], 0.0)
            for k in range(4):
                t_ = spool.tile([128, BL], F32R, tag=f"hT{k}")
                nc.vector.tensor_copy(t_[:], zinit[:])
                hT.append(t_)
            h_sb = spool.tile([BL, HID], F32, tag="h_sb")
            nc.gpsimd.memset(h_sb[:], 0.0)

            for t in range(gru_steps):
                gxt = gxtp.tile([BL, H3], F32R, tag="gxt")
                nc.sync.dma_start(out=gxt[:], in_=gx_d[t])
                # psum preloads: gx terms via identity matmul, b_hh_n via ones
                pr = ps_g.tile([BL, HID], F32, tag="pg")
                pz = ps_g.tile([BL, HID], F32, tag="pg")
                pn = ps_g.tile([BL, HID], F32, tag="pg")
                px = ps_g.tile([BL, HID], F32, tag="pg")
                nc.tensor.matmul(out=pr[:], lhsT=i8[:], rhs=gxt[:, 0:HID],
                                 start=True, stop=False)
                nc.tensor.matmul(out=pz[:], lhsT=i8[:], rhs=gxt[:, HID:2 * HID],
                                 start=True, stop=False)
                nc.tensor.matmul(out=px[:], lhsT=i8[:], rhs=gxt[:, 2 * HID:3 * HID],
                                 start=True, stop=True)
                nc.tensor.matmul(out=pn[:], lhsT=ones1r[:, :BL],
                                 rhs=bhh_sb[:, 2 * HID:3 * HID],
                                 start=True, stop=False)
                for gi, pg in enumerate((pr, pz, pn)):
                    for k in range(4):
                        nc.tensor.matmul(
                            out=pg[:], lhsT=hT[k][:],
                            rhs=whht_sb[k][:, HID * gi:HID * (gi + 1)],
                            start=False, stop=(k == 3))
                # gate tail in 256-wide halves; h'-chunks release next step's
                # K-chunk matmuls early
                for sh in range(2):
                    cs = slice(256 * sh, 256 * (sh + 1))
                    rh = work.tile([BL, 256], F32, tag="rh")
                    zh = work.tile([BL, 256], F32, tag="zh")
                    nh = work.tile([BL, 256], F32, tag="nh")
                    th = work.tile([BL, 256], F32, tag="th")
                    nc.scalar.activation(rh[:], pr[:, cs], AF.Sigmoid)
                    nc.scalar.activation(zh[:], pz[:, cs], AF.Sigmoid)
                    nc.vector.tensor_mul(th[:], rh[:], pn[:, cs])
                    nc.vector.tensor_add(th[:], th[:], px[:, cs])
                    nc.scalar.activation(nh[:], th[:], AF.Tanh)
                    # h' = n + z * (h - n)
                    nc.vector.tensor_sub(th[:], h_sb[:, cs], nh[:])
                    nc.vector.tensor_mul(th[:], zh[:], th[:])
                    nc.vector.tensor_add(h_sb[:, cs], nh[:], th[:])
                    if t < gru_steps - 1:
                        for c in (2 * sh, 2 * sh + 1):
                            pst = ps_t.tile([128, 128], F32, tag="pst")
                            nc.tensor.transpose(
                                out=pst[:128, :BL],
                                in_=h_sb[:, 128 * c:128 * (c + 1)],
                                identity=ident[:BL, :BL])
                            nc.vector.tensor_copy(hT[c][:], pst[:128, :BL])
                nc.sync.dma_start(out=outs_d[t * BL:(t + 1) * BL, :], in_=h_sb[:])

            # ================= phase 4: last, attention, context =================
            last_sb = spool.tile([BL, HID], F32, tag="last_sb")
            nc.gpsimd.indirect_dma_start(
                out=last_sb[:], out_offset=None, in_=outs_d[:],
                in_offset=bass.IndirectOffsetOnAxis(ap=ixl[:, :1], axis=0))
            lastT = []
            for k in range(4):
                pst = ps_t.tile([128, 128], F32, tag="pst")
                nc.tensor.transpose(out=pst[:128, :BL],
                                    in_=last_sb[:, 128 * k:128 * (k + 1)],
                                    identity=ident[:BL, :BL])
                lk = spool.tile([128, BL], F32R, tag=f"lastT{k}")
                nc.vector.tensor_copy(lk[:], pst[:128, :BL])
                lastT.append(lk)
            # energies + softmax, per batch on partition 0; weights written
            # straight into w_row [1, 2048]
            w_row = spool.tile([1, NGL], F32R, tag="w_row")
            for b in range(BL):
                pse = ps_b.tile([1, NG], F32, tag="psb", name=f"pse{b}")
                for m in range(4):
                    nc.tensor.matmul(
                        out=pse[:], lhsT=lastT[m][:, b:b + 1],
                        rhs=histT[m][:, NG * b:NG * (b + 1)],
                        start=(m == 0), stop=(m == 3))
                mxb = spool.tile([1, 1], F32, tag="mxb", name=f"mxb{b}")
                nc.vector.tensor_reduce(mxb[:], pse[:], axis=AX.X, op=ALU.max)
                nc.vector.tensor_scalar_mul(mxb[:], mxb[:], -1.0)
                exb = spool.tile([1, NG], F32, tag="exb", name=f"exb{b}")
                nc.scalar.activation(exb[:], pse[:], AF.Exp, bias=mxb[:, 0:1])
                smb = spool.tile([1, 1], F32, tag="smb", name=f"smb{b}")
                nc.vector.tensor_reduce(smb[:], exb[:], axis=AX.X, op=ALU.add)
                rsb = spool.tile([1, 1], F32, tag="rsb", name=f"rsb{b}")
                nc.vector.reciprocal(rsb[:], smb[:])
                nc.scalar.mul(w_row[0:1, NG * b:NG * (b + 1)], exb[:], rsb[:, 0:1])
            og = spool.tile([BL, H3], F32, tag="og")
            nc.vector.tensor_copy(og[:, 0:HID], last_sb[:])
            nc.vector.tensor_copy(og[:, 2 * HID:3 * HID], last_sb[:])
            ctxT = [spool.tile([128, BL], F32, tag=f"ctxT{m}", name=f"ctxT{m}")
                    for m in range(4)]
            for s in range(NGL // 512):
                psw = ps_b.tile([128, 512], F32, tag="psb")
                nc.tensor.matmul(out=psw[:], lhsT=ones1r[:],
                                 rhs=w_row[0:1, 512 * s:512 * (s + 1)],
                                 start=True, stop=True)
                for m in range(4):
                    whc = work.tile([128, 512], F32, tag="whc")
                    nc.vector.tensor_mul(
                        whc[:], histT[m][:, 512 * s:512 * (s + 1)].bitcast(F32),
                        psw[:])
                    nc.vector.tensor_reduce(
                        ctxT[m][:, 2 * s:2 * s + 2],
                        whc[:].rearrange("p (b g) -> p b g", b=2),
                        axis=AX.X, op=ALU.add)
            for m in range(4):
                pst = ps_t.tile([128, 128], F32, tag="pst")
                nc.tensor.transpose(out=pst[:BL, :128], in_=ctxT[m][:],
                                    identity=ident[:])
                nc.vector.tensor_copy(og[:, HID + 128 * m:HID + 128 * (m + 1)],
                                      pst[:BL, :128])
            nc.sync.dma_start(out=og_local[:], in_=og[:])

            # ================= phase 5: allgather + final matmul =================
            if sim:
                # timing-sim stand-in (TimelineSim can't model collectives):
                # replicate local og into all 8 slots
                for c in range(NCORES):
                    nc.sync.dma_start(out=og_shared[BL * c:BL * (c + 1), :],
                                      in_=og[:])
            else:
                nc.gpsimd.collective_compute(
                    "AllGather", ALU.bypass, replica_groups=[list(range(NCORES))],
                    ins=[og_local[:]], outs=[og_shared[:]])
            ogf = spool.tile([B, H3], F32, tag="ogf")
            nc.sync.dma_start(out=ogf[:], in_=og_shared[:])
            outT = []
            for k in range(12):
                pst = ps_t.tile([128, 128], F32, tag="pst")
                nc.tensor.transpose(out=pst[:128, :B],
                                    in_=ogf[:, 128 * k:128 * (k + 1)],
                                    identity=ident[:B, :B])
                ok_ = wpool.tile([128, B], F32R, tag=f"outT{k}")
                nc.vector.tensor_copy(ok_[:], pst[:128, :B])
                outT.append(ok_)
            y_sb = spool.tile([B, ULP], F32, tag="y_sb")
            for n, (c0, cn) in enumerate(((0, 512), (512, ULP - 512))):
                psy = ps_b.tile([B, cn], F32, tag="psb")
                for k in range(12):
                    nc.tensor.matmul(out=psy[:], lhsT=outT[k][:],
                                     rhs=wfin_sb[k][:, c0:c0 + cn],
                                     start=(k == 0), stop=False)
                nc.tensor.matmul(out=psy[:], lhsT=ones1r[:, :B],
                                 rhs=wfin_b[:, c0:c0 + cn],
                                 start=False, stop=True)
                nc.vector.tensor_copy(y_sb[:, c0:c0 + cn], psy[:])
            for t in range(T):
                nc.sync.dma_start(out=score[:, t, :], in_=y_sb[:, 0:UL])

    nc.compile()
    return nc


def _collect_in_maps(inputs):
    loc = np.asarray(inputs["loc"]).astype(np.int32)
    tim = np.asarray(inputs["tim"]).astype(np.int32)
    lens = np.asarray(inputs["input_lengths"]).astype(np.int32)
    hloc = np.asarray(inputs["history_loc"]).astype(np.int32)
    htim = np.asarray(inputs["history_tim"]).astype(np.int32)
    huid = np.asarray(inputs["history_uid"]).astype(np.int32)
    gsz = int(np.asarray(inputs["group_size"]))
    assert gsz == G
    emb_loc = np.ascontiguousarray(np.asarray(inputs["emb_loc"], dtype=np.float32))
    emb_tim = np.ascontiguousarray(np.asarray(inputs["emb_tim"], dtype=np.float32))
    emb_uid = np.ascontiguousarray(np.asarray(inputs["emb_uid"], dtype=np.float32))
    W_attn = np.asarray(inputs["W_attn"], dtype=np.float32)
    b_attn = np.asarray(inputs["b_attn"], dtype=np.float32)
    W_ih = np.asarray(inputs["W_ih"], dtype=np.float32)
    b_ih = np.asarray(inputs["b_ih"], dtype=np.float32)
    W_hh = np.asarray(inputs["W_hh"], dtype=np.float32)
    b_hh = np.asarray(inputs["b_hh"], dtype=np.float32)
    W_final = np.asarray(inputs["W_final"], dtype=np.float32)
    b_final = np.asarray(inputs["b_final"], dtype=np.float32)

    bias_row = b_ih.copy()
    bias_row[0:2 * HID] += b_hh[0:2 * HID]
    wiht = np.ascontiguousarray(
        np.vstack([W_ih.T, bias_row[None, :]]).astype(np.float32))
    whht = np.ascontiguousarray(W_hh.T.astype(np.float32))
    bhh = np.ascontiguousarray(b_hh[None, :].astype(np.float32))
    watt = np.ascontiguousarray(
        np.vstack([W_attn.T, b_attn[None, :]]).astype(np.float32))
    smean = np.zeros((128, 32), np.float32)
    for i in range(128):
        smean[i, i // 4] = 0.25

    in_maps = []
    for c in range(NCORES):
        bs = slice(c * BL, (c + 1) * BL)
        wfin = np.zeros((H3 + 1, ULP), np.float32)
        wfin[:H3, :UL] = W_final[c * UL:(c + 1) * UL, :].T
        wfin[H3, :UL] = b_final[c * UL:(c + 1) * UL]
        idx_last = ((lens[bs] - 1) * BL + np.arange(BL, dtype=np.int32))[:, None]
        in_maps.append({
            "idx_cur": np.ascontiguousarray(loc[bs].reshape(TOK // 128, 128).T),
            "idx_tim": np.ascontiguousarray(tim[bs].reshape(TOK // 128, 128).T),
            "idx_hloc": np.ascontiguousarray(hloc[bs, ::G].reshape(NGL // 128, 128).T),
            "idx_htim": np.ascontiguousarray(htim[bs, ::G].reshape(NGL // 128, 128).T),
            "idx_huid": np.ascontiguousarray(huid[bs].reshape(BL * L_HIST // 128, 128).T),
            "idx_last": np.ascontiguousarray(idx_last.astype(np.int32)),
            "emb_loc": emb_loc, "emb_tim": emb_tim, "emb_uid": emb_uid,
            "wiht": wiht, "whht": whht, "bhh": bhh, "watt": watt,
            "wfin": wfin, "smean": smean,
            "onesd": np.ones((1, 128), np.float32),
            "eye8": np.eye(BL, dtype=np.float32),
        })

    return in_maps


def kernel(**inputs):
    in_maps = _collect_in_maps(inputs)
    if "nc" not in _STATE:
        _STATE["nc"] = _build()
    res = run_bass_kernel_spmd(_STATE["nc"], in_maps, list(range(NCORES))).results
    return np.concatenate([res[c]["score"] for c in range(NCORES)], axis=2)


def run_traced(inputs):
    """Dev helper: same run but with NTFF tracing; returns BassKernelResults."""
    in_maps = _collect_in_maps(inputs)
    if "nc" not in _STATE:
        _STATE["nc"] = _build()
    return run_bass_kernel_spmd(_STATE["nc"], in_maps, list(range(NCORES)), trace=True)


# revision 19
# speedup vs baseline: 1.0887x; 1.0887x over previous
"""Trainium2 Bass kernel for nn_AttnRnnModel (GRU + history attention + big output).

Sharding (8 cores):
  - batch-sharded (8 batches/core): embedding gathers, gx precompute, GRU
    recurrence, history pooling + attention context
  - AllGather of [last|context|last] (8x1536 -> 64x1536)
  - U-sharded final matmul (625 vocab columns/core) + broadcast-over-T output
    write of the core's [64, 128, 625] slice; host concatenates along U.

All heavy matmuls run as float32r (full-rate on PE, ~1e-5 rel precision).
Weight transposes (W_ih.T etc.) are done host-side; data-dependent
transposes (gathered embeddings, h_t, attention features) on the PE.
"""

import numpy as np
import concourse.bass as bass
import concourse.bacc as bacc
import concourse.mybir as mybir
from concourse import tile
from concourse.bass_utils import run_bass_kernel_spmd
from concourse.masks import make_identity

F32 = mybir.dt.float32
F32R = mybir.dt.float32r
I32 = mybir.dt.int32
AF = mybir.ActivationFunctionType
ALU = mybir.AluOpType
AX = mybir.AxisListType

# problem constants (hardcoded per harness contract)
LOC_SIZE, LOC_EMB = 100000, 256
TIM_SIZE, TIM_EMB = 48, 64
UID_SIZE, UID_EMB = 5000, 128
HID = 512
B, T = 64, 128
L_HIST, G = 1024, 4
NCORES = 8
BL = B // NCORES              # 8 batches per core
NG = L_HIST // G              # 256 history groups per batch
NGL = BL * NG                 # 2048 groups per core
DIN = LOC_EMB + TIM_EMB       # 320
DH = LOC_EMB + TIM_EMB + UID_EMB  # 448
H3 = 3 * HID                  # 1536
UL = UID_SIZE // NCORES       # 625 vocab columns per core
ULP = UL + 1                  # padded even width for fp32r matmuls
TOK = BL * T                  # 1024 current-traj tokens per core

_STATE = {}


def _build(sim=False, gru_steps=T, skip_hist=False, skip_p1=False):
    nc = bacc.Bacc("TRN2", target_bir_lowering=False, debug=False,
                   num_devices=NCORES)

    # ---- per-core external inputs (host pre-arranged) ----
    idx_cur = nc.declare_dram_parameter("idx_cur", [128, TOK // 128], I32, isOutput=False)
    idx_tim = nc.declare_dram_parameter("idx_tim", [128, TOK // 128], I32, isOutput=False)
    idx_hloc = nc.declare_dram_parameter("idx_hloc", [128, NGL // 128], I32, isOutput=False)
    idx_htim = nc.declare_dram_parameter("idx_htim", [128, NGL // 128], I32, isOutput=False)
    idx_huid = nc.declare_dram_parameter("idx_huid", [128, BL * L_HIST // 128], I32, isOutput=False)
    idx_last = nc.declare_dram_parameter("idx_last", [BL, 1], I32, isOutput=False)

    emb_loc = nc.declare_dram_parameter("emb_loc", [LOC_SIZE, LOC_EMB], F32, isOutput=False)
    emb_tim = nc.declare_dram_parameter("emb_tim", [TIM_SIZE, TIM_EMB], F32, isOutput=False)
    emb_uid = nc.declare_dram_parameter("emb_uid", [UID_SIZE, UID_EMB], F32, isOutput=False)

    wiht = nc.declare_dram_parameter("wiht", [DIN + 1, H3], F32R, isOutput=False)   # [W_ih.T; b_ih]
    whht = nc.declare_dram_parameter("whht", [HID, H3], F32R, isOutput=False)       # W_hh.T
    bhh = nc.declare_dram_parameter("bhh", [1, H3], F32R, isOutput=False)
    watt = nc.declare_dram_parameter("watt", [DH + 1, HID], F32R, isOutput=False)   # [W_attn.T; b_attn]
    wfin = nc.declare_dram_parameter("wfin", [H3 + 1, ULP], F32R, isOutput=False)    # [W_final.T slice; b_final slice]
    smean = nc.declare_dram_parameter("smean", [128, 32], F32R, isOutput=False)      # group-mean matrix
    onesd = nc.declare_dram_parameter("onesd", [1, 128], F32R, isOutput=False)
    eye8 = nc.declare_dram_parameter("eye8", [BL, BL], F32R, isOutput=False)

    score = nc.declare_dram_parameter("score", [B, T, UL], F32, isOutput=True)

    # ---- internal DRAM ----
    gx_d = nc.dram_tensor("gx_d", [T, BL, H3], F32R)
    outs_d = nc.dram_tensor("outs_d", [T * BL, HID], F32)
    og_local = nc.dram_tensor("og_local", [BL, H3], F32)
    og_shared = nc.dram_tensor("og_shared", [B, H3], F32, addr_space="Shared")

    FEAT_CH = ((0, 128), (128, 128), (256, 128), (384, 65))

    with tile.TileContext(nc) as tc:
        with (
            tc.tile_pool(name="const", bufs=1) as cpool,
            tc.tile_pool(name="wts", bufs=1) as wpool,
            tc.tile_pool(name="state", bufs=1) as spool,
            tc.tile_pool(name="work", bufs=2) as work,
            tc.tile_pool(name="gxt", bufs=2) as gxtp,
            tc.tile_pool(name="p2", bufs=2) as p2,
            tc.tile_pool(name="ps_g", bufs=4, space="PSUM") as ps_g,
            tc.tile_pool(name="ps_t", bufs=2, space="PSUM") as ps_t,
            tc.tile_pool(name="ps_b", bufs=2, space="PSUM") as ps_b,
        ):
            ident = cpool.tile([128, 128], F32, tag="ident")
            make_identity(nc, ident[:])
            ones1r = cpool.tile([1, 128], F32R, tag="ones1r")
            nc.sync.dma_start(out=ones1r[:], in_=onesd[:])
            i8 = cpool.tile([BL, BL], F32R, tag="i8")
            nc.sync.dma_start(out=i8[:], in_=eye8[:])

            # ---- load persistent weights ----
            whht_sb = []
            for k in range(4):
                t_ = wpool.tile([128, H3], F32R, tag=f"whht{k}")
                nc.sync.dma_start(out=t_[:], in_=whht[128 * k:128 * (k + 1), :])
                whht_sb.append(t_)
            bhh_sb = wpool.tile([1, H3], F32R, tag="bhh")
            nc.sync.dma_start(out=bhh_sb[:], in_=bhh[:])
            watt_sb = []
            for k, (p0, pn) in enumerate(FEAT_CH):
                t_ = wpool.tile([pn, HID], F32R, tag=f"watt{k}")
                nc.sync.dma_start(out=t_[:], in_=watt[p0:p0 + pn, :])
                watt_sb.append(t_)
            smean_sb = cpool.tile([128, 32], F32R, tag="smean")
            nc.sync.dma_start(out=smean_sb[:], in_=smean[:])

            # ---- load index tiles ----
            def idx_tile(name, src, ncol):
                t_ = cpool.tile([128, ncol], I32, tag=name)
                nc.sync.dma_start(out=t_[:], in_=src[:])
                return t_
            ixc = idx_tile("ixc", idx_cur, TOK // 128)
            ixt = idx_tile("ixt", idx_tim, TOK // 128)
            ixhl = idx_tile("ixhl", idx_hloc, NGL // 128)
            ixht = idx_tile("ixht", idx_htim, NGL // 128)
            ixhu = idx_tile("ixhu", idx_huid, BL * L_HIST // 128)
            ixl = cpool.tile([BL, 1], I32, tag="ixl")
            nc.sync.dma_start(out=ixl[:], in_=idx_last[:])

            # histT[m] = tanh(featT.T-ish @ W_attn) in [HID-chunk, groups] layout
            histT = []
            for m in range(4):
                t_ = wpool.tile([128, NGL], F32R, tag=f"histT{m}")
                histT.append(t_)

            # ================= phase 1: x gather + transpose + gx =================
            with (tc.tile_pool(name="p1", bufs=2) as p1,
                  tc.tile_pool(name="p1w", bufs=1) as p1w):
                wiht_sb = []
                for k, (p0, pn) in enumerate(((0, 128), (128, 128), (256, 65))):
                    wk = p1w.tile([pn, H3], F32R, tag=f"wih{k}")
                    nc.sync.dma_start(out=wk[:], in_=wiht[p0:p0 + pn, :])
                    wiht_sb.append(wk)
                for b in range(0 if skip_p1 else BL):
                    x_t = p1.tile([128, DIN], F32, tag="x_t")
                    nc.gpsimd.indirect_dma_start(
                        out=x_t[:, 0:LOC_EMB], out_offset=None, in_=emb_loc[:],
                        in_offset=bass.IndirectOffsetOnAxis(ap=ixc[:, b:b + 1], axis=0))
                    nc.gpsimd.indirect_dma_start(
                        out=x_t[:, LOC_EMB:DIN], out_offset=None, in_=emb_tim[:],
                        in_offset=bass.IndirectOffsetOnAxis(ap=ixt[:, b:b + 1], axis=0))
                    xT = []
                    for k, (c0, cn) in enumerate(((0, 128), (128, 128), (256, 64))):
                        pst = ps_t.tile([128, 128], F32, tag="pst")
                        nc.tensor.transpose(out=pst[:cn, :128], in_=x_t[:, c0:c0 + cn],
                                            identity=ident[:])
                        xk = p1.tile([cn + (1 if k == 2 else 0), 128], F32R, tag=f"xT{k}")
                        nc.vector.tensor_copy(xk[:cn, :], pst[:cn, :128])
                        if k == 2:
                            nc.sync.dma_start(out=xk[cn:cn + 1, :], in_=onesd[:])
                        xT.append(xk)
                    for n in range(3):
                        psg = ps_b.tile([128, HID], F32, tag="psb")
                        for k in range(3):
                            nc.tensor.matmul(
                                out=psg[:], lhsT=xT[k][:],
                                rhs=wiht_sb[k][:, HID * n:HID * (n + 1)],
                                start=(k == 0), stop=(k == 2))
                        gxc = p1.tile([128, HID], F32R, tag="gxc")
                        nc.vector.tensor_copy(gxc[:], psg[:])
                        nc.sync.dma_start(out=gx_d[:, b, HID * n:HID * (n + 1)],
                                          in_=gxc[:])

            # ================= phase 2: history features -> histT =================
            # emitted as units interleaved into the GRU loop so the PE never
            # idles long enough for HAM to re-throttle.
            fTc_store = {}

            def hist_gather(gc, fi):
                f = 4 * gc + fi
                ft = p2.tile([128, DH + 1], F32, tag="ft", name=f"ft{f}")
                nc.gpsimd.indirect_dma_start(
                    out=ft[:, 0:LOC_EMB], out_offset=None, in_=emb_loc[:],
                    in_offset=bass.IndirectOffsetOnAxis(ap=ixhl[:, f:f + 1], axis=0))
                nc.gpsimd.indirect_dma_start(
                    out=ft[:, LOC_EMB:DIN], out_offset=None, in_=emb_tim[:],
                    in_offset=bass.IndirectOffsetOnAxis(ap=ixht[:, f:f + 1], axis=0))
                nc.gpsimd.memset(ft[:, DH:DH + 1], 1.0)
                ut = p2.tile([128, 4 * UID_EMB], F32R, tag="ut", name=f"ut{f}")
                for q in range(4):
                    nc.gpsimd.indirect_dma_start(
                        out=ut[:, UID_EMB * q:UID_EMB * (q + 1)], out_offset=None,
                        in_=emb_uid[:],
                        in_offset=bass.IndirectOffsetOnAxis(
                            ap=ixhu[:, 4 * f + q:4 * f + q + 1], axis=0))
                return ft, ut

            def hist_compute(gc, fi, ft, ut):
                if fi == 0:
                    fTc_store[gc] = [
                        p2.tile([cn, 512], F32R, tag=f"fTc{k}", name=f"fTc{k}_{gc}")
                        for k, (c0, cn) in enumerate(FEAT_CH)]
                fTc = fTc_store[gc]
                psm = ps_b.tile([32, 4 * UID_EMB], F32, tag="psb", name=f"psm{gc}_{fi}")
                nc.tensor.matmul(out=psm[:], lhsT=smean_sb[:], rhs=ut[:],
                                 start=True, stop=True)
                for q in range(4):
                    nc.vector.tensor_copy(ft[32 * q:32 * (q + 1), DIN:DH],
                                          psm[:32, UID_EMB * q:UID_EMB * (q + 1)])
                for k, (c0, cn) in enumerate(FEAT_CH):
                    pst = ps_t.tile([128, 128], F32, tag="pst")
                    nc.tensor.transpose(out=pst[:cn, :128], in_=ft[:, c0:c0 + cn],
                                        identity=ident[:])
                    nc.vector.tensor_copy(fTc[k][:, 128 * fi:128 * (fi + 1)],
                                          pst[:cn, :128])

            def hist_mm(gc):
                fTc = fTc_store[gc]
                for m in range(4):
                    psh = ps_b.tile([128, 512], F32, tag="psb", name=f"psh{gc}_{m}")
                    for k in range(4):
                        nc.tensor.matmul(
                            out=psh[:], lhsT=watt_sb[k][:, 128 * m:128 * (m + 1)],
                            rhs=fTc[k][:], start=(k == 0), stop=(k == 3))
                    nc.scalar.activation(histT[m][:, 512 * gc:512 * (gc + 1)],
                                         psh[:], AF.Tanh)

            hist_actions = []
            if not skip_hist:
                for gc in range(4):
                    pend = {}
                    acts = []
                    # g0 g1 c0 g2 c1 g3 c2 c3 h  (gather >=1 slot before compute)
                    order = [("g", 0), ("g", 1), ("c", 0), ("g", 2), ("c", 1),
                             ("g", 3), ("c", 2), ("c", 3), ("h", None)]
                    for kind, fi in order:
                        hist_actions.append((kind, gc, fi))

            def run_hist_action(state, action):
                kind, gc, fi = action
                if kind == "g":
                    state[(gc, fi)] = hist_gather(gc, fi)
                elif kind == "c":
                    ft, ut = state.pop((gc, fi))
                    hist_compute(gc, fi, ft, ut)
                else:
                    hist_mm(gc)

            # ================= phase 3: GRU over T steps =================
            hTbig = spool.tile([128, 4 * BL], F32R, tag="hTbig")
            zinit = spool.tile([128, 4 * BL], F32, tag="zinit")
            nc.gpsimd.memset(zinit[:], 0.0)
            nc.vector.tensor_copy(hTbig[:], zinit[:])
            h_sb = spool.tile([BL, HID], F32, tag="h_sb")
            nc.gpsimd.memset(h_sb[:], 0.0)

            hstate = {}
            hist_q = list(hist_actions)
            for t in range(gru_steps):
                gxt = gxtp.tile([BL, H3], F32R, tag="gxt")
                nc.sync.dma_start(out=gxt[:], in_=gx_d[t])
                # psum preloads: gx terms via identity matmul, b_hh_n via ones
                pr = ps_g.tile([BL, HID], F32, tag="pg")
                pz = ps_g.tile([BL, HID], F32, tag="pg")
                pn = ps_g.tile([BL, HID], F32, tag="pg")
                px = ps_g.tile([BL, HID], F32, tag="pg")
                nc.tensor.matmul(out=pr[:], lhsT=i8[:], rhs=gxt[:, 0:HID],
                                 start=True, stop=False)
                nc.tensor.matmul(out=pz[:], lhsT=i8[:], rhs=gxt[:, HID:2 * HID],
                                 start=True, stop=False)
                nc.tensor.matmul(out=px[:], lhsT=i8[:], rhs=gxt[:, 2 * HID:3 * HID],
                                 start=True, stop=True)
                nc.tensor.matmul(out=pn[:], lhsT=ones1r[:, :BL],
                                 rhs=bhh_sb[:, 2 * HID:3 * HID],
                                 start=True, stop=False)
                for gi, pg in enumerate((pr, pz, pn)):
                    for k in range(4):
                        nc.tensor.matmul(
                            out=pg[:], lhsT=hTbig[:, BL * k:BL * (k + 1)],
                            rhs=whht_sb[k][:, HID * gi:HID * (gi + 1)],
                            start=False, stop=(k == 3))
                # one history unit every 3rd step fills the PE during the tail
                if t % 3 == 1 and hist_q:
                    run_hist_action(hstate, hist_q.pop(0))
                # gate tail in 256-wide halves; h'-chunks release next step's
                # K-chunk matmuls early
                for sh in range(2):
                    cs = slice(256 * sh, 256 * (sh + 1))
                    rh = work.tile([BL, 256], F32, tag="rh")
                    zh = work.tile([BL, 256], F32, tag="zh")
                    nh = work.tile([BL, 256], F32, tag="nh")
                    th = work.tile([BL, 256], F32, tag="th")
                    nc.scalar.activation(rh[:], pr[:, cs], AF.Sigmoid)
                    nc.scalar.activation(zh[:], pz[:, cs], AF.Sigmoid)
                    nc.vector.tensor_mul(th[:], rh[:], pn[:, cs])
                    nc.vector.tensor_add(th[:], th[:], px[:, cs])
                    nc.scalar.activation(nh[:], th[:], AF.Tanh)
                    # h' = n + z * (h - n)
                    nc.vector.tensor_sub(th[:], h_sb[:, cs], nh[:])
                    nc.vector.tensor_mul(th[:], zh[:], th[:])
                    nc.vector.tensor_add(h_sb[:, cs], nh[:], th[:])
                    if t < gru_steps - 1:
                        pst = ps_t.tile([128, 128], F32, tag="pst")
                        for j, c in enumerate((2 * sh, 2 * sh + 1)):
                            nc.tensor.transpose(
                                out=pst[:128, BL * j:BL * (j + 1)],
                                in_=h_sb[:, 128 * c:128 * (c + 1)],
                                identity=ident[:BL, :BL])
                        nc.vector.tensor_copy(
                            hTbig[:, 2 * BL * sh:2 * BL * (sh + 1)],
                            pst[:128, :2 * BL])
                nc.sync.dma_start(out=outs_d[t * BL:(t + 1) * BL, :], in_=h_sb[:])
            while hist_q:
                run_hist_action(hstate, hist_q.pop(0))

            # ================= phase 4: last, attention, context =================
            last_sb = spool.tile([BL, HID], F32, tag="last_sb")
            nc.gpsimd.indirect_dma_start(
                out=last_sb[:], out_offset=None, in_=outs_d[:],
                in_offset=bass.IndirectOffsetOnAxis(ap=ixl[:, :1], axis=0))
            lastT = []
            for k in range(4):
                pst = ps_t.tile([128, 128], F32, tag="pst")
                nc.tensor.transpose(out=pst[:128, :BL],
                                    in_=last_sb[:, 128 * k:128 * (k + 1)],
                                    identity=ident[:BL, :BL])
                lk = spool.tile([128, BL], F32R, tag=f"lastT{k}")
                nc.vector.tensor_copy(lk[:], pst[:128, :BL])
                lastT.append(lk)
            # energies + softmax, per batch on partition 0; weights written
            # straight into w_row [1, 2048]
            w_row = spool.tile([1, NGL], F32R, tag="w_row")
            for b in range(BL):
                pse = ps_b.tile([1, NG], F32, tag="psb", name=f"pse{b}")
                for m in range(4):
                    nc.tensor.matmul(
                        out=pse[:], lhsT=lastT[m][:, b:b + 1],
                        rhs=histT[m][:, NG * b:NG * (b + 1)],
                        start=(m == 0), stop=(m == 3))
                mxb = spool.tile([1, 1], F32, tag="mxb", name=f"mxb{b}")
                nc.vector.tensor_reduce(mxb[:], pse[:], axis=AX.X, op=ALU.max)
                nc.vector.tensor_scalar_mul(mxb[:], mxb[:], -1.0)
                exb = spool.tile([1, NG], F32, tag="exb", name=f"exb{b}")
                nc.scalar.activation(exb[:], pse[:], AF.Exp, bias=mxb[:, 0:1])
                smb = spool.tile([1, 1], F32, tag="smb", name=f"smb{b}")
                nc.vector.tensor_reduce(smb[:], exb[:], axis=AX.X, op=ALU.add)
                rsb = spool.tile([1, 1], F32, tag="rsb", name=f"rsb{b}")
                nc.vector.reciprocal(rsb[:], smb[:])
                nc.scalar.mul(w_row[0:1, NG * b:NG * (b + 1)], exb[:], rsb[:, 0:1])
            og = spool.tile([BL, H3], F32, tag="og")
            nc.vector.tensor_copy(og[:, 0:HID], last_sb[:])
            nc.vector.tensor_copy(og[:, 2 * HID:3 * HID], last_sb[:])
            ctxT = [spool.tile([128, BL], F32, tag=f"ctxT{m}", name=f"ctxT{m}")
                    for m in range(4)]
            for s in range(NGL // 512):
                psw = ps_b.tile([128, 512], F32, tag="psb")
                nc.tensor.matmul(out=psw[:], lhsT=ones1r[:],
                                 rhs=w_row[0:1, 512 * s:512 * (s + 1)],
                                 start=True, stop=True)
                for m in range(4):
                    whc = work.tile([128, 512], F32, tag="whc")
                    nc.vector.tensor_mul(
                        whc[:], histT[m][:, 512 * s:512 * (s + 1)].bitcast(F32),
                        psw[:])
                    nc.vector.tensor_reduce(
                        ctxT[m][:, 2 * s:2 * s + 2],
                        whc[:].rearrange("p (b g) -> p b g", b=2),
                        axis=AX.X, op=ALU.add)
            for m in range(4):
                pst = ps_t.tile([128, 128], F32, tag="pst")
                nc.tensor.transpose(out=pst[:BL, :128], in_=ctxT[m][:],
                                    identity=ident[:])
                nc.vector.tensor_copy(og[:, HID + 128 * m:HID + 128 * (m + 1)],
                                      pst[:BL, :128])
            nc.sync.dma_start(out=og_local[:], in_=og[:])

            # ================= phase 5: allgather + final matmul =================
            p5cm = tc.tile_pool(name="p5", bufs=1)
            p5 = p5cm.__enter__()
            wfin_sb = []
            for k in range(12):
                t_ = p5.tile([128, ULP], F32R, tag=f"wfin{k}", name=f"wfin{k}")
                nc.sync.dma_start(out=t_[:], in_=wfin[128 * k:128 * (k + 1), :])
                wfin_sb.append(t_)
            wfin_b = p5.tile([1, ULP], F32R, tag="wfinb")
            nc.sync.dma_start(out=wfin_b[:], in_=wfin[H3:H3 + 1, :])
            if sim:
                # timing-sim stand-in (TimelineSim can't model collectives):
                # replicate local og into all 8 slots
                for c in range(NCORES):
                    nc.sync.dma_start(out=og_shared[BL * c:BL * (c + 1), :],
                                      in_=og[:])
            else:
                nc.gpsimd.collective_compute(
                    "AllGather", ALU.bypass, replica_groups=[list(range(NCORES))],
                    ins=[og_local[:]], outs=[og_shared[:]])
            ogf = spool.tile([B, H3], F32, tag="ogf")
            nc.sync.dma_start(out=ogf[:], in_=og_shared[:])
            outT = []
            for k in range(12):
                pst = ps_t.tile([128, 128], F32, tag="pst")
                nc.tensor.transpose(out=pst[:128, :B],
                                    in_=ogf[:, 128 * k:128 * (k + 1)],
                                    identity=ident[:B, :B])
                ok_ = wpool.tile([128, B], F32R, tag=f"outT{k}")
                nc.vector.tensor_copy(ok_[:], pst[:128, :B])
                outT.append(ok_)
            y_sb = spool.tile([B, ULP], F32, tag="y_sb")
            for n, (c0, cn) in enumerate(((0, 512), (512, ULP - 512))):
                psy = ps_b.tile([B, cn], F32, tag="psb")
                for k in range(12):
                    nc.tensor.matmul(out=psy[:], lhsT=outT[k][:],
                                     rhs=wfin_sb[k][:, c0:c0 + cn],
                                     start=(k == 0), stop=False)
                nc.tensor.matmul(out=psy[:], lhsT=ones1r[:, :B],
                                 rhs=wfin_b[:, c0:c0 + cn],
                                 start=False, stop=True)
                nc.vector.tensor_copy(y_sb[:, c0:c0 + cn], psy[:])
            for t in range(T):
                nc.sync.dma_start(out=score[:, t, :], in_=y_sb[:, 0:UL])
            p5cm.__exit__(None, None, None)

    nc.compile()
    return nc


def _collect_in_maps(inputs):
    loc = np.asarray(inputs["loc"]).astype(np.int32)
    tim = np.asarray(inputs["tim"]).astype(np.int32)
    lens = np.asarray(inputs["input_lengths"]).astype(np.int32)
    hloc = np.asarray(inputs["history_loc"]).astype(np.int32)
    htim = np.asarray(inputs["history_tim"]).astype(np.int32)
    huid = np.asarray(inputs["history_uid"]).astype(np.int32)
    gsz = int(np.asarray(inputs["group_size"]))
    assert gsz == G
    emb_loc = np.ascontiguousarray(np.asarray(inputs["emb_loc"], dtype=np.float32))
    emb_tim = np.ascontiguousarray(np.asarray(inputs["emb_tim"], dtype=np.float32))
    emb_uid = np.ascontiguousarray(np.asarray(inputs["emb_uid"], dtype=np.float32))
    W_attn = np.asarray(inputs["W_attn"], dtype=np.float32)
    b_attn = np.asarray(inputs["b_attn"], dtype=np.float32)
    W_ih = np.asarray(inputs["W_ih"], dtype=np.float32)
    b_ih = np.asarray(inputs["b_ih"], dtype=np.float32)
    W_hh = np.asarray(inputs["W_hh"], dtype=np.float32)
    b_hh = np.asarray(inputs["b_hh"], dtype=np.float32)
    W_final = np.asarray(inputs["W_final"], dtype=np.float32)
    b_final = np.asarray(inputs["b_final"], dtype=np.float32)

    bias_row = b_ih.copy()
    bias_row[0:2 * HID] += b_hh[0:2 * HID]
    wiht = np.ascontiguousarray(
        np.vstack([W_ih.T, bias_row[None, :]]).astype(np.float32))
    whht = np.ascontiguousarray(W_hh.T.astype(np.float32))
    bhh = np.ascontiguousarray(b_hh[None, :].astype(np.float32))
    watt = np.ascontiguousarray(
        np.vstack([W_attn.T, b_attn[None, :]]).astype(np.float32))
    smean = np.zeros((128, 32), np.float32)
    for i in range(128):
        smean[i, i // 4] = 0.25

    in_maps = []
    for c in range(NCORES):
        bs = slice(c * BL, (c + 1) * BL)
        wfin = np.zeros((H3 + 1, ULP), np.float32)
        wfin[:H3, :UL] = W_final[c * UL:(c + 1) * UL, :].T
        wfin[H3, :UL] = b_final[c * UL:(c + 1) * UL]
        idx_last = ((lens[bs] - 1) * BL + np.arange(BL, dtype=np.int32))[:, None]
        in_maps.append({
            "idx_cur": np.ascontiguousarray(loc[bs].reshape(TOK // 128, 128).T),
            "idx_tim": np.ascontiguousarray(tim[bs].reshape(TOK // 128, 128).T),
            "idx_hloc": np.ascontiguousarray(hloc[bs, ::G].reshape(NGL // 128, 128).T),
            "idx_htim": np.ascontiguousarray(htim[bs, ::G].reshape(NGL // 128, 128).T),
            "idx_huid": np.ascontiguousarray(huid[bs].reshape(BL * L_HIST // 128, 128).T),
            "idx_last": np.ascontiguousarray(idx_last.astype(np.int32)),
            "emb_loc": emb_loc, "emb_tim": emb_tim, "emb_uid": emb_uid,
            "wiht": wiht, "whht": whht, "bhh": bhh, "watt": watt,
            "wfin": wfin, "smean": smean,
            "onesd": np.ones((1, 128), np.float32),
            "eye8": np.eye(BL, dtype=np.float32),
        })

    return in_maps


def kernel(**inputs):
    in_maps = _collect_in_maps(inputs)
    if "nc" not in _STATE:
        _STATE["nc"] = _build()
    res = run_bass_kernel_spmd(_STATE["nc"], in_maps, list(range(NCORES))).results
    return np.concatenate([res[c]["score"] for c in range(NCORES)], axis=2)


def run_traced(inputs, tmpdir=None):
    """Dev helper: same run but with NTFF tracing; returns BassKernelResults."""
    in_maps = _collect_in_maps(inputs)
    if "nc" not in _STATE:
        _STATE["nc"] = _build()
    return run_bass_kernel_spmd(_STATE["nc"], in_maps, list(range(NCORES)),
                                trace=True, tmpdir=tmpdir)


# revision 22
# speedup vs baseline: 1.1230x; 1.0315x over previous
"""Trainium2 Bass kernel for nn_AttnRnnModel (GRU + history attention + big output).

Sharding (8 cores):
  - batch-sharded (8 batches/core): embedding gathers, gx precompute, GRU
    recurrence, history pooling + attention context
  - AllGather of [last|context|last] (8x1536 -> 64x1536)
  - U-sharded final matmul (625 vocab columns/core) + broadcast-over-T output
    write of the core's [64, 128, 625] slice; host concatenates along U.

All heavy matmuls run as float32r (full-rate on PE, ~1e-5 rel precision).
Weight transposes (W_ih.T etc.) are done host-side; data-dependent
transposes (gathered embeddings, h_t, attention features) on the PE.
"""

import numpy as np
import concourse.bass as bass
import concourse.bacc as bacc
import concourse.mybir as mybir
from concourse import tile
from concourse.bass_utils import run_bass_kernel_spmd
from concourse.masks import make_identity

F32 = mybir.dt.float32
F32R = mybir.dt.float32r
I32 = mybir.dt.int32
AF = mybir.ActivationFunctionType
ALU = mybir.AluOpType
AX = mybir.AxisListType

# problem constants (hardcoded per harness contract)
LOC_SIZE, LOC_EMB = 100000, 256
TIM_SIZE, TIM_EMB = 48, 64
UID_SIZE, UID_EMB = 5000, 128
HID = 512
B, T = 64, 128
L_HIST, G = 1024, 4
NCORES = 8
BL = B // NCORES              # 8 batches per core
NG = L_HIST // G              # 256 history groups per batch
NGL = BL * NG                 # 2048 groups per core
DIN = LOC_EMB + TIM_EMB       # 320
DH = LOC_EMB + TIM_EMB + UID_EMB  # 448
H3 = 3 * HID                  # 1536
UL = UID_SIZE // NCORES       # 625 vocab columns per core
ULP = UL + 1                  # padded even width for fp32r matmuls
TOK = BL * T                  # 1024 current-traj tokens per core

_STATE = {}


def _build(sim=False, gru_steps=T, skip_hist=False, skip_p1=False):
    nc = bacc.Bacc("TRN2", target_bir_lowering=False, debug=False,
                   num_devices=NCORES)

    # ---- per-core external inputs (host pre-arranged) ----
    idx_cur = nc.declare_dram_parameter("idx_cur", [128, TOK // 128], I32, isOutput=False)
    idx_tim = nc.declare_dram_parameter("idx_tim", [128, TOK // 128], I32, isOutput=False)
    idx_hloc = nc.declare_dram_parameter("idx_hloc", [128, NGL // 128], I32, isOutput=False)
    idx_htim = nc.declare_dram_parameter("idx_htim", [128, NGL // 128], I32, isOutput=False)
    idx_huid = nc.declare_dram_parameter("idx_huid", [128, BL * L_HIST // 128], I32, isOutput=False)
    idx_last = nc.declare_dram_parameter("idx_last", [BL, 1], I32, isOutput=False)

    emb_loc = nc.declare_dram_parameter("emb_loc", [LOC_SIZE, LOC_EMB], F32, isOutput=False)
    emb_tim = nc.declare_dram_parameter("emb_tim", [TIM_SIZE, TIM_EMB], F32, isOutput=False)
    emb_uid = nc.declare_dram_parameter("emb_uid", [UID_SIZE, UID_EMB], F32, isOutput=False)

    wiht = nc.declare_dram_parameter("wiht", [DIN + 1, H3], F32R, isOutput=False)   # [W_ih.T; b_ih]
    whht = nc.declare_dram_parameter("whht", [HID, H3], F32R, isOutput=False)       # W_hh.T
    bhh = nc.declare_dram_parameter("bhh", [1, H3], F32R, isOutput=False)
    watt = nc.declare_dram_parameter("watt", [DH + 1, HID], F32R, isOutput=False)   # [W_attn.T; b_attn]
    wfin = nc.declare_dram_parameter("wfin", [H3 + 1, ULP], F32R, isOutput=False)    # [W_final.T slice; b_final slice]
    smean = nc.declare_dram_parameter("smean", [128, 32], F32R, isOutput=False)      # group-mean matrix
    onesd = nc.declare_dram_parameter("onesd", [1, 128], F32R, isOutput=False)
    eye8 = nc.declare_dram_parameter("eye8", [BL, BL], F32R, isOutput=False)

    score = nc.declare_dram_parameter("score", [B, T, UL], F32, isOutput=True)

    # ---- internal DRAM ----
    gx_d = nc.dram_tensor("gx_d", [T, BL, H3], F32R)
    outs_d = nc.dram_tensor("outs_d", [T * BL, HID], F32)
    og_local = nc.dram_tensor("og_local", [BL, H3], F32)
    og_shared = nc.dram_tensor("og_shared", [B, H3], F32, addr_space="Shared")

    FEAT_CH = ((0, 128), (128, 128), (256, 128), (384, 65))

    with tile.TileContext(nc) as tc:
        with (
            tc.tile_pool(name="const", bufs=1) as cpool,
            tc.tile_pool(name="wts", bufs=1) as wpool,
            tc.tile_pool(name="state", bufs=1) as spool,
            tc.tile_pool(name="work", bufs=2) as work,
            tc.tile_pool(name="gxt", bufs=2) as gxtp,
            tc.tile_pool(name="p2", bufs=2) as p2,
            tc.tile_pool(name="ps_g", bufs=3, space="PSUM") as ps_g,
            tc.tile_pool(name="ps_t", bufs=2, space="PSUM") as ps_t,
            tc.tile_pool(name="ps_b", bufs=2, space="PSUM") as ps_b,
        ):
            ident = cpool.tile([128, 128], F32, tag="ident")
            make_identity(nc, ident[:])
            ones1r = cpool.tile([1, 128], F32R, tag="ones1r")
            nc.sync.dma_start(out=ones1r[:], in_=onesd[:])
            i8 = cpool.tile([BL, BL], F32R, tag="i8")
            nc.sync.dma_start(out=i8[:], in_=eye8[:])

            # ---- load persistent weights ----
            whht_sb = []
            for k in range(4):
                t_ = wpool.tile([128, H3], F32R, tag=f"whht{k}")
                nc.sync.dma_start(out=t_[:], in_=whht[128 * k:128 * (k + 1), :])
                whht_sb.append(t_)
            bhh_sb = wpool.tile([1, H3], F32R, tag="bhh")
            nc.sync.dma_start(out=bhh_sb[:], in_=bhh[:])
            watt_sb = []
            for k, (p0, pn) in enumerate(FEAT_CH):
                t_ = wpool.tile([pn, HID], F32R, tag=f"watt{k}")
                nc.sync.dma_start(out=t_[:], in_=watt[p0:p0 + pn, :])
                watt_sb.append(t_)
            smean_sb = cpool.tile([128, 32], F32R, tag="smean")
            nc.sync.dma_start(out=smean_sb[:], in_=smean[:])

            # ---- load index tiles ----
            def idx_tile(name, src, ncol):
                t_ = cpool.tile([128, ncol], I32, tag=name)
                nc.sync.dma_start(out=t_[:], in_=src[:])
                return t_
            ixc = idx_tile("ixc", idx_cur, TOK // 128)
            ixt = idx_tile("ixt", idx_tim, TOK // 128)
            ixhl = idx_tile("ixhl", idx_hloc, NGL // 128)
            ixht = idx_tile("ixht", idx_htim, NGL // 128)
            ixhu = idx_tile("ixhu", idx_huid, BL * L_HIST // 128)
            ixl = cpool.tile([BL, 1], I32, tag="ixl")
            nc.sync.dma_start(out=ixl[:], in_=idx_last[:])

            # histT[m] = tanh(featT.T-ish @ W_attn) in [HID-chunk, groups] layout
            histT = []
            for m in range(4):
                t_ = wpool.tile([128, NGL], F32R, tag=f"histT{m}")
                histT.append(t_)

            # ================= phase 1: x gather + transpose + gx =================
            with (tc.tile_pool(name="p1", bufs=2) as p1,
                  tc.tile_pool(name="p1w", bufs=1) as p1w):
                wiht_sb = []
                for k, (p0, pn) in enumerate(((0, 128), (128, 128), (256, 65))):
                    wk = p1w.tile([pn, H3], F32R, tag=f"wih{k}")
                    nc.sync.dma_start(out=wk[:], in_=wiht[p0:p0 + pn, :])
                    wiht_sb.append(wk)
                for b in range(0 if skip_p1 else BL):
                    x_t = p1.tile([128, DIN], F32, tag="x_t")
                    nc.gpsimd.indirect_dma_start(
                        out=x_t[:, 0:LOC_EMB], out_offset=None, in_=emb_loc[:],
                        in_offset=bass.IndirectOffsetOnAxis(ap=ixc[:, b:b + 1], axis=0))
                    nc.gpsimd.indirect_dma_start(
                        out=x_t[:, LOC_EMB:DIN], out_offset=None, in_=emb_tim[:],
                        in_offset=bass.IndirectOffsetOnAxis(ap=ixt[:, b:b + 1], axis=0))
                    xT = []
                    for k, (c0, cn) in enumerate(((0, 128), (128, 128), (256, 64))):
                        pst = ps_t.tile([128, 128], F32, tag="pst")
                        nc.tensor.transpose(out=pst[:cn, :128], in_=x_t[:, c0:c0 + cn],
                                            identity=ident[:])
                        xk = p1.tile([cn + (1 if k == 2 else 0), 128], F32R, tag=f"xT{k}")
                        nc.vector.tensor_copy(xk[:cn, :], pst[:cn, :128])
                        if k == 2:
                            nc.sync.dma_start(out=xk[cn:cn + 1, :], in_=onesd[:])
                        xT.append(xk)
                    for n in range(3):
                        psg = ps_b.tile([128, HID], F32, tag="psb")
                        for k in range(3):
                            nc.tensor.matmul(
                                out=psg[:], lhsT=xT[k][:],
                                rhs=wiht_sb[k][:, HID * n:HID * (n + 1)],
                                start=(k == 0), stop=(k == 2))
                        gxc = p1.tile([128, HID], F32R, tag="gxc")
                        nc.vector.tensor_copy(gxc[:], psg[:])
                        nc.sync.dma_start(out=gx_d[:, b, HID * n:HID * (n + 1)],
                                          in_=gxc[:])

            # ================= phase 2: history features -> histT =================
            # emitted as units interleaved into the GRU loop so the PE never
            # idles long enough for HAM to re-throttle.
            fTc_store = {}

            def hist_gather(gc, fi):
                f = 4 * gc + fi
                ft = p2.tile([128, DH + 1], F32, tag="ft", name=f"ft{f}")
                nc.gpsimd.indirect_dma_start(
                    out=ft[:, 0:LOC_EMB], out_offset=None, in_=emb_loc[:],
                    in_offset=bass.IndirectOffsetOnAxis(ap=ixhl[:, f:f + 1], axis=0))
                nc.gpsimd.indirect_dma_start(
                    out=ft[:, LOC_EMB:DIN], out_offset=None, in_=emb_tim[:],
                    in_offset=bass.IndirectOffsetOnAxis(ap=ixht[:, f:f + 1], axis=0))
                nc.gpsimd.memset(ft[:, DH:DH + 1], 1.0)
                ut = p2.tile([128, 4 * UID_EMB], F32R, tag="ut", name=f"ut{f}")
                for q in range(4):
                    nc.gpsimd.indirect_dma_start(
                        out=ut[:, UID_EMB * q:UID_EMB * (q + 1)], out_offset=None,
                        in_=emb_uid[:],
                        in_offset=bass.IndirectOffsetOnAxis(
                            ap=ixhu[:, 4 * f + q:4 * f + q + 1], axis=0))
                return ft, ut

            def hist_compute(gc, fi, ft, ut):
                if fi == 0:
                    fTc_store[gc] = [
                        p2.tile([cn, 512], F32R, tag=f"fTc{k}", name=f"fTc{k}_{gc}")
                        for k, (c0, cn) in enumerate(FEAT_CH)]
                fTc = fTc_store[gc]
                psm = ps_b.tile([32, 4 * UID_EMB], F32, tag="psb", name=f"psm{gc}_{fi}")
                nc.tensor.matmul(out=psm[:], lhsT=smean_sb[:], rhs=ut[:],
                                 start=True, stop=True)
                for q in range(4):
                    nc.vector.tensor_copy(ft[32 * q:32 * (q + 1), DIN:DH],
                                          psm[:32, UID_EMB * q:UID_EMB * (q + 1)])
                for k, (c0, cn) in enumerate(FEAT_CH):
                    pst = ps_t.tile([128, 128], F32, tag="pst")
                    nc.tensor.transpose(out=pst[:cn, :128], in_=ft[:, c0:c0 + cn],
                                        identity=ident[:])
                    nc.vector.tensor_copy(fTc[k][:, 128 * fi:128 * (fi + 1)],
                                          pst[:cn, :128])

            def hist_mm(gc):
                fTc = fTc_store[gc]
                for m in range(4):
                    psh = ps_b.tile([128, 512], F32, tag="psb", name=f"psh{gc}_{m}")
                    for k in range(4):
                        nc.tensor.matmul(
                            out=psh[:], lhsT=watt_sb[k][:, 128 * m:128 * (m + 1)],
                            rhs=fTc[k][:], start=(k == 0), stop=(k == 3))
                    nc.scalar.activation(histT[m][:, 512 * gc:512 * (gc + 1)],
                                         psh[:], AF.Tanh)

            hist_actions = []
            if not skip_hist:
                for gc in range(4):
                    pend = {}
                    acts = []
                    # g0 g1 c0 g2 c1 g3 c2 c3 h  (gather >=1 slot before compute)
                    order = [("g", 0), ("g", 1), ("c", 0), ("g", 2), ("c", 1),
                             ("g", 3), ("c", 2), ("c", 3), ("h", None)]
                    for kind, fi in order:
                        hist_actions.append((kind, gc, fi))

            def run_hist_action(state, action):
                kind, gc, fi = action
                if kind == "g":
                    state[(gc, fi)] = hist_gather(gc, fi)
                elif kind == "c":
                    ft, ut = state.pop((gc, fi))
                    hist_compute(gc, fi, ft, ut)
                else:
                    hist_mm(gc)

            # ================= phase 3: GRU over T steps =================
            hTbig = spool.tile([128, 4 * BL], F32R, tag="hTbig")
            zinit = spool.tile([128, 4 * BL], F32, tag="zinit")
            nc.gpsimd.memset(zinit[:], 0.0)
            nc.vector.tensor_copy(hTbig[:], zinit[:])
            h_sb = spool.tile([BL, HID], F32, tag="h_sb")
            nc.gpsimd.memset(h_sb[:], 0.0)

            hstate = {}
            hist_q = list(hist_actions)
            for t in range(gru_steps):
                gxt = gxtp.tile([BL, H3], F32R, tag="gxt")
                nc.sync.dma_start(out=gxt[:], in_=gx_d[t])
                # psum preloads: gx terms via identity matmul, b_hh_n via ones
                pr = ps_g.tile([BL, HID], F32, tag="pg")
                pz = ps_g.tile([BL, HID], F32, tag="pg")
                pn = ps_g.tile([BL, HID], F32, tag="pg")
                nc.tensor.matmul(out=pr[:], lhsT=i8[:], rhs=gxt[:, 0:HID],
                                 start=True, stop=False)
                nc.tensor.matmul(out=pz[:], lhsT=i8[:], rhs=gxt[:, HID:2 * HID],
                                 start=True, stop=False)
                nc.tensor.matmul(out=pn[:], lhsT=ones1r[:, :BL],
                                 rhs=bhh_sb[:, 2 * HID:3 * HID],
                                 start=True, stop=False)
                for gi, pg in enumerate((pr, pz, pn)):
                    for k in range(4):
                        nc.tensor.matmul(
                            out=pg[:], lhsT=hTbig[:, BL * k:BL * (k + 1)],
                            rhs=whht_sb[k][:, HID * gi:HID * (gi + 1)],
                            start=False, stop=(k == 3))
                # one history unit every 3rd step fills the PE during the tail
                if t % 3 == 1 and hist_q:
                    run_hist_action(hstate, hist_q.pop(0))
                # gate tail in 256-wide halves; h'-chunks release next step's
                # K-chunk matmuls early
                for sh in range(2):
                    cs = slice(256 * sh, 256 * (sh + 1))
                    rh = work.tile([BL, 256], F32, tag="rh")
                    zh = work.tile([BL, 256], F32, tag="zh")
                    nh = work.tile([BL, 256], F32, tag="nh")
                    th = work.tile([BL, 256], F32, tag="th")
                    nc.scalar.activation(rh[:], pr[:, cs], AF.Sigmoid)
                    nc.scalar.activation(zh[:], pz[:, cs], AF.Sigmoid)
                    nc.vector.tensor_mul(th[:], rh[:], pn[:, cs])
                    nc.vector.tensor_add(th[:], th[:],
                                         gxt[:, 2 * HID + 256 * sh:
                                             2 * HID + 256 * (sh + 1)].bitcast(F32))
                    nc.scalar.activation(nh[:], th[:], AF.Tanh)
                    # h' = n + z * (h - n)
                    nc.vector.tensor_sub(th[:], h_sb[:, cs], nh[:])
                    nc.vector.tensor_mul(th[:], zh[:], th[:])
                    nc.vector.tensor_add(h_sb[:, cs], nh[:], th[:])
                    if t < gru_steps - 1:
                        pst = ps_t.tile([128, 128], F32, tag="pst")
                        for j, c in enumerate((2 * sh, 2 * sh + 1)):
                            nc.tensor.transpose(
                                out=pst[:128, BL * j:BL * (j + 1)],
                                in_=h_sb[:, 128 * c:128 * (c + 1)],
                                identity=ident[:BL, :BL])
                        nc.vector.tensor_copy(
                            hTbig[:, 2 * BL * sh:2 * BL * (sh + 1)],
                            pst[:128, :2 * BL])
                    # HAM keep-warm fillers: real matmuls, result discarded
                    if t < gru_steps - 1:
                        pfill = ps_t.tile([BL, 128], F32, tag="pfill", bufs=1)
                        for fj in range(5):
                            nc.tensor.matmul(
                                out=pfill[:], lhsT=i8[:],
                                rhs=gxt[:, 128 * fj:128 * (fj + 1)],
                                start=True, stop=True)
                nc.sync.dma_start(out=outs_d[t * BL:(t + 1) * BL, :], in_=h_sb[:])
            while hist_q:
                run_hist_action(hstate, hist_q.pop(0))

            # ================= phase 4: last, attention, context =================
            last_sb = spool.tile([BL, HID], F32, tag="last_sb")
            nc.gpsimd.indirect_dma_start(
                out=last_sb[:], out_offset=None, in_=outs_d[:],
                in_offset=bass.IndirectOffsetOnAxis(ap=ixl[:, :1], axis=0))
            lastT = []
            for k in range(4):
                pst = ps_t.tile([128, 128], F32, tag="pst")
                nc.tensor.transpose(out=pst[:128, :BL],
                                    in_=last_sb[:, 128 * k:128 * (k + 1)],
                                    identity=ident[:BL, :BL])
                lk = spool.tile([128, BL], F32R, tag=f"lastT{k}")
                nc.vector.tensor_copy(lk[:], pst[:128, :BL])
                lastT.append(lk)
            # energies + softmax, per batch on partition 0; weights written
            # straight into w_row [1, 2048]
            w_row = spool.tile([1, NGL], F32R, tag="w_row")
            for b in range(BL):
                pse = ps_b.tile([1, NG], F32, tag="psb", name=f"pse{b}")
                for m in range(4):
                    nc.tensor.matmul(
                        out=pse[:], lhsT=lastT[m][:, b:b + 1],
                        rhs=histT[m][:, NG * b:NG * (b + 1)],
                        start=(m == 0), stop=(m == 3))
                mxb = spool.tile([1, 1], F32, tag="mxb", name=f"mxb{b}")
                nc.vector.tensor_reduce(mxb[:], pse[:], axis=AX.X, op=ALU.max)
                nc.vector.tensor_scalar_mul(mxb[:], mxb[:], -1.0)
                exb = spool.tile([1, NG], F32, tag="exb", name=f"exb{b}")
                nc.scalar.activation(exb[:], pse[:], AF.Exp, bias=mxb[:, 0:1])
                smb = spool.tile([1, 1], F32, tag="smb", name=f"smb{b}")
                nc.vector.tensor_reduce(smb[:], exb[:], axis=AX.X, op=ALU.add)
                rsb = spool.tile([1, 1], F32, tag="rsb", name=f"rsb{b}")
                nc.vector.reciprocal(rsb[:], smb[:])
                nc.scalar.mul(w_row[0:1, NG * b:NG * (b + 1)], exb[:], rsb[:, 0:1])
            og = spool.tile([BL, H3], F32, tag="og")
            nc.vector.tensor_copy(og[:, 0:HID], last_sb[:])
            nc.vector.tensor_copy(og[:, 2 * HID:3 * HID], last_sb[:])
            ctxT = [spool.tile([128, BL], F32, tag=f"ctxT{m}", name=f"ctxT{m}")
                    for m in range(4)]
            for s in range(NGL // 512):
                psw = ps_b.tile([128, 512], F32, tag="psb")
                nc.tensor.matmul(out=psw[:], lhsT=ones1r[:],
                                 rhs=w_row[0:1, 512 * s:512 * (s + 1)],
                                 start=True, stop=True)
                for m in range(4):
                    whc = work.tile([128, 512], F32, tag="whc")
                    nc.vector.tensor_mul(
                        whc[:], histT[m][:, 512 * s:512 * (s + 1)].bitcast(F32),
                        psw[:])
                    nc.vector.tensor_reduce(
                        ctxT[m][:, 2 * s:2 * s + 2],
                        whc[:].rearrange("p (b g) -> p b g", b=2),
                        axis=AX.X, op=ALU.add)
            for m in range(4):
                pst = ps_t.tile([128, 128], F32, tag="pst")
                nc.tensor.transpose(out=pst[:BL, :128], in_=ctxT[m][:],
                                    identity=ident[:])
                nc.vector.tensor_copy(og[:, HID + 128 * m:HID + 128 * (m + 1)],
                                      pst[:BL, :128])
            nc.sync.dma_start(out=og_local[:], in_=og[:])

            # ================= phase 5: allgather + final matmul =================
            p5cm = tc.tile_pool(name="p5", bufs=1)
            p5 = p5cm.__enter__()
            wfin_sb = []
            for k in range(12):
                t_ = p5.tile([128, ULP], F32R, tag=f"wfin{k}", name=f"wfin{k}")
                nc.sync.dma_start(out=t_[:], in_=wfin[128 * k:128 * (k + 1), :])
                wfin_sb.append(t_)
            wfin_b = p5.tile([1, ULP], F32R, tag="wfinb")
            nc.sync.dma_start(out=wfin_b[:], in_=wfin[H3:H3 + 1, :])
            if sim:
                # timing-sim stand-in (TimelineSim can't model collectives):
                # replicate local og into all 8 slots
                for c in range(NCORES):
                    nc.sync.dma_start(out=og_shared[BL * c:BL * (c + 1), :],
                                      in_=og[:])
            else:
                nc.gpsimd.collective_compute(
                    "AllGather", ALU.bypass, replica_groups=[list(range(NCORES))],
                    ins=[og_local[:]], outs=[og_shared[:]])
            ogf = spool.tile([B, H3], F32, tag="ogf")
            nc.sync.dma_start(out=ogf[:], in_=og_shared[:])
            outT = []
            for k in range(12):
                pst = ps_t.tile([128, 128], F32, tag="pst")
                nc.tensor.transpose(out=pst[:128, :B],
                                    in_=ogf[:, 128 * k:128 * (k + 1)],
                                    identity=ident[:B, :B])
                ok_ = wpool.tile([128, B], F32R, tag=f"outT{k}")
                nc.vector.tensor_copy(ok_[:], pst[:128, :B])
                outT.append(ok_)
            y_sb = spool.tile([B, ULP], F32, tag="y_sb")
            for n, (c0, cn) in enumerate(((0, 512), (512, ULP - 512))):
                psy = ps_b.tile([B, cn], F32, tag="psb")
                for k in range(12):
                    nc.tensor.matmul(out=psy[:], lhsT=outT[k][:],
                                     rhs=wfin_sb[k][:, c0:c0 + cn],
                                     start=(k == 0), stop=False)
                nc.tensor.matmul(out=psy[:], lhsT=ones1r[:, :B],
                                 rhs=wfin_b[:, c0:c0 + cn],
                                 start=False, stop=True)
                nc.vector.tensor_copy(y_sb[:, c0:c0 + cn], psy[:])
            for t in range(T):
                nc.sync.dma_start(out=score[:, t, :], in_=y_sb[:, 0:UL])
            p5cm.__exit__(None, None, None)

    nc.compile()
    return nc


def _collect_in_maps(inputs):
    loc = np.asarray(inputs["loc"]).astype(np.int32)
    tim = np.asarray(inputs["tim"]).astype(np.int32)
    lens = np.asarray(inputs["input_lengths"]).astype(np.int32)
    hloc = np.asarray(inputs["history_loc"]).astype(np.int32)
    htim = np.asarray(inputs["history_tim"]).astype(np.int32)
    huid = np.asarray(inputs["history_uid"]).astype(np.int32)
    gsz = int(np.asarray(inputs["group_size"]))
    assert gsz == G
    emb_loc = np.ascontiguousarray(np.asarray(inputs["emb_loc"], dtype=np.float32))
    emb_tim = np.ascontiguousarray(np.asarray(inputs["emb_tim"], dtype=np.float32))
    emb_uid = np.ascontiguousarray(np.asarray(inputs["emb_uid"], dtype=np.float32))
    W_attn = np.asarray(inputs["W_attn"], dtype=np.float32)
    b_attn = np.asarray(inputs["b_attn"], dtype=np.float32)
    W_ih = np.asarray(inputs["W_ih"], dtype=np.float32)
    b_ih = np.asarray(inputs["b_ih"], dtype=np.float32)
    W_hh = np.asarray(inputs["W_hh"], dtype=np.float32)
    b_hh = np.asarray(inputs["b_hh"], dtype=np.float32)
    W_final = np.asarray(inputs["W_final"], dtype=np.float32)
    b_final = np.asarray(inputs["b_final"], dtype=np.float32)

    bias_row = b_ih.copy()
    bias_row[0:2 * HID] += b_hh[0:2 * HID]
    wiht = np.ascontiguousarray(
        np.vstack([W_ih.T, bias_row[None, :]]).astype(np.float32))
    whht = np.ascontiguousarray(W_hh.T.astype(np.float32))
    bhh = np.ascontiguousarray(b_hh[None, :].astype(np.float32))
    watt = np.ascontiguousarray(
        np.vstack([W_attn.T, b_attn[None, :]]).astype(np.float32))
    smean = np.zeros((128, 32), np.float32)
    for i in range(128):
        smean[i, i // 4] = 0.25

    in_maps = []
    for c in range(NCORES):
        bs = slice(c * BL, (c + 1) * BL)
        wfin = np.zeros((H3 + 1, ULP), np.float32)
        wfin[:H3, :UL] = W_final[c * UL:(c + 1) * UL, :].T
        wfin[H3, :UL] = b_final[c * UL:(c + 1) * UL]
        idx_last = ((lens[bs] - 1) * BL + np.arange(BL, dtype=np.int32))[:, None]
        in_maps.append({
            "idx_cur": np.ascontiguousarray(loc[bs].reshape(TOK // 128, 128).T),
            "idx_tim": np.ascontiguousarray(tim[bs].reshape(TOK // 128, 128).T),
            "idx_hloc": np.ascontiguousarray(hloc[bs, ::G].reshape(NGL // 128, 128).T),
            "idx_htim": np.ascontiguousarray(htim[bs, ::G].reshape(NGL // 128, 128).T),
            "idx_huid": np.ascontiguousarray(huid[bs].reshape(BL * L_HIST // 128, 128).T),
            "idx_last": np.ascontiguousarray(idx_last.astype(np.int32)),
            "emb_loc": emb_loc, "emb_tim": emb_tim, "emb_uid": emb_uid,
            "wiht": wiht, "whht": whht, "bhh": bhh, "watt": watt,
            "wfin": wfin, "smean": smean,
            "onesd": np.ones((1, 128), np.float32),
            "eye8": np.eye(BL, dtype=np.float32),
        })

    return in_maps


def kernel(**inputs):
    in_maps = _collect_in_maps(inputs)
    if "nc" not in _STATE:
        _STATE["nc"] = _build()
    res = run_bass_kernel_spmd(_STATE["nc"], in_maps, list(range(NCORES))).results
    return np.concatenate([res[c]["score"] for c in range(NCORES)], axis=2)


def run_traced(inputs, tmpdir=None):
    """Dev helper: same run but with NTFF tracing; returns BassKernelResults."""
    in_maps = _collect_in_maps(inputs)
    if "nc" not in _STATE:
        _STATE["nc"] = _build()
    return run_bass_kernel_spmd(_STATE["nc"], in_maps, list(range(NCORES)),
                                trace=True, tmpdir=tmpdir)
